# revision 17
# baseline (speedup 1.0000x reference)
"""Trainium2 Bass kernel for nn_BatchedBitNetFFN (BitNet b1.58 batched-expert FFN).

Math per expert (reference semantics reproduced exactly up to fp ulp noise):
  xq   = fake-quant(x): per-token int8 absmax  -> xq_int in [-127,127] (exact in bf16)
  wq   = ternary quant: per-expert scale mean|w| -> wq_int in {-1,0,1} (exact in bf16)
  gate = xq @ wq_g^T ; up = xq @ wq_u^T        (exact integer GEMMs, f32 PSUM accum)
  h    = sigmoid(gate*scale) * up
  hq   = fake-quant(h) per-token over F
  out  = (hq @ wq_d^T) * folded_scales

Sharding: expert-parallel, E=16 experts -> 2 experts on each of 8 NeuronCores,
no cross-core communication. Rounding uses the +/- 1.5*2^23 magic constant
(exact round-to-nearest-even, matching jnp.round). All transposed (K-on-
partition) operand layouts are produced by the HW xbar DMA-transpose on bf16.
"""

import numpy as np

E_FULL, C_FULL, D, F = 16, 4096, 768, 2048
NCORES = 8
EPC = E_FULL // NCORES  # experts per core
MAGIC = 12582912.0  # 1.5 * 2**23 -> exact RNE integer rounding via add/sub

_cache = {}


def emit(tc, x_d, wg_d, wu_d, wd_d, out_d, epc, C):
    import concourse.mybir as mybir

    nc = tc.nc
    f32 = mybir.dt.float32
    bf16 = mybir.dt.bfloat16
    AX = mybir.AxisListType.X
    OP = mybir.AluOpType
    AF = mybir.ActivationFunctionType

    DT = D // 128   # 6  d-chunks
    FT = F // 128   # 16 f-chunks
    FG = F // 512   # 4  f-groups (N=512)
    CG = C // 512   # c-groups per expert
    CPG = 4         # c-chunks (of 128 tokens) per c-group

    from contextlib import ExitStack
    ctx = ExitStack()
    tc._emit_ctx = ctx  # keep pools alive until TileContext exit

    const_p = ctx.enter_context(tc.tile_pool(name="const", bufs=1))
    wld_p = ctx.enter_context(tc.tile_pool(name="wld", bufs=3))
    stats_p = ctx.enter_context(tc.tile_pool(name="stats", bufs=3))
    small_p = ctx.enter_context(tc.tile_pool(name="small", bufs=8))
    wqs_p = ctx.enter_context(tc.tile_pool(name="wqs", bufs=1))
    wgT_p = ctx.enter_context(tc.tile_pool(name="wgTp", bufs=1))
    wuT_p = ctx.enter_context(tc.tile_pool(name="wuTp", bufs=1))
    wdT_p = ctx.enter_context(tc.tile_pool(name="wdTp", bufs=1))
    xld_p = ctx.enter_context(tc.tile_pool(name="xld", bufs=3))
    xqs_p = ctx.enter_context(tc.tile_pool(name="xqs", bufs=2))
    xqT_p = ctx.enter_context(tc.tile_pool(name="xqTp", bufs=2))
    sg_p = ctx.enter_context(tc.tile_pool(name="sgp", bufs=3))
    t3_p = ctx.enter_context(tc.tile_pool(name="t3p", bufs=3))
    hq_p = ctx.enter_context(tc.tile_pool(name="hqp", bufs=2))
    hqT_p = ctx.enter_context(tc.tile_pool(name="hqTp", bufs=3))
    out_p = ctx.enter_context(tc.tile_pool(name="outp", bufs=2))

    gp_p = ctx.enter_context(tc.tile_pool(name="gpp", bufs=2, space="PSUM"))
    up_p = ctx.enter_context(tc.tile_pool(name="upp", bufs=2, space="PSUM"))
    op_p = ctx.enter_context(tc.tile_pool(name="opp", bufs=3, space="PSUM"))
    bc_p = ctx.enter_context(tc.tile_pool(name="bcp", bufs=1, space="PSUM"))

    ones_k = const_p.tile([128, 1], f32, name="ones_k")
    nc.vector.memset(ones_k[:], 1.0)
    ones_m = const_p.tile([1, 128], f32, name="ones_m")
    nc.vector.memset(ones_m[:], 1.0)

    # weight descriptors: (dram, n 128-row tiles, row length, load tag)
    wspecs = [
        (wg_d, F // 128, D, "wta"),
        (wu_d, F // 128, D, "wta"),
        (wd_d, D // 128, F, "wtb"),
    ]

    for e in range(epc):
        # ---------- weight passes, per weight: scale then quantize ----------
        # bcast cols: 2j=sw_j  2j+1=rsw_j  6=kb(=swu*swd/127)
        bcps = bc_p.tile([128, 8], f32, tag="bcps", name=f"bcps{e}")
        svals = stats_p.tile([1, 8], f32, tag="svals", name=f"svals{e}")
        bcast = stats_p.tile([128, 8], f32, tag="bcast", name=f"bcast{e}")
        wTs = []
        for widx, (w_d, ntile, fd, wtag) in enumerate(wspecs):
            # pass 1: sw = clip(mean|w|, 1e-5)
            parts = stats_p.tile([128, ntile], f32, tag="parts", name=f"parts{e}_{widx}")
            for t in range(ntile):
                wt = wld_p.tile([128, fd], f32, tag=wtag, name=f"wt{wtag}",
                                bufs=(2 if widx == 2 else 3))
                nc.sync.dma_start(wt[:], w_d[e, 128 * t:128 * (t + 1), :])
                nc.scalar.activation(wt[:], wt[:], AF.Abs,
                                     accum_out=parts[:, t:t + 1])
            colsum = small_p.tile([128, 1], f32, name="colsum")
            nc.vector.tensor_reduce(colsum[:], parts[:], axis=AX, op=OP.add)
            # cross-partition sum -> [1,1]
            nc.tensor.matmul(bcps[0:1, 7:8], colsum[:], ones_k[:],
                             start=True, stop=True)
            sw = svals[:, 2 * widx:2 * widx + 1]
            nc.scalar.mul(sw, bcps[0:1, 7:8], 1.0 / (F * D))
            nc.vector.tensor_scalar_max(sw, sw, 1e-5)
            nc.vector.reciprocal(svals[:, 2 * widx + 1:2 * widx + 2], sw)
            # broadcast (sw, rsw) across partitions right away
            nc.tensor.matmul(bcps[:, 2 * widx:2 * widx + 2], ones_m[:],
                             svals[:, 2 * widx:2 * widx + 2],
                             start=True, stop=True)
            nc.scalar.copy(bcast[:, 2 * widx:2 * widx + 2],
                           bcps[:, 2 * widx:2 * widx + 2])

            # pass 2: ternary quantize -> bf16, xbar-transposed
            if widx < 2:
                wT = [wgT_p, wuT_p][widx].tile(
                    [128, FT, DT, 128], bf16, tag="wT",
                    name=f"w{'gu'[widx]}T{e}")
            else:
                wT = wdT_p.tile([128, DT, FT, 128], bf16, tag="wT",
                                name=f"wdT{e}")
            rsw = bcast[:, 2 * widx + 1:2 * widx + 2]
            grp = 2 if widx == 2 else 4
            for h in range(ntile // grp):
                wq_stage = wqs_p.tile([128, grp, fd], bf16, tag="wqs",
                                      name=f"wqs{e}_{widx}_{h}")
                for tt in range(grp):
                    t = h * grp + tt
                    wt = wld_p.tile([128, fd], f32, tag=wtag, name=f"wt{wtag}",
                                    bufs=(2 if widx == 2 else 3))
                    nc.sync.dma_start(wt[:], w_d[e, 128 * t:128 * (t + 1), :])
                    # round(clip(w/s,-1,1)) == clip(round(w/s),-1,1); RNE magic
                    nc.vector.tensor_scalar(wt[:], wt[:], rsw, MAGIC,
                                            OP.mult, OP.add)
                    nc.gpsimd.tensor_scalar(wt[:], wt[:], MAGIC, 1.0,
                                            OP.subtract, OP.min)
                    nc.gpsimd.tensor_scalar(wq_stage[:, tt, :], wt[:], -1.0,
                                            None, OP.max)
                nc.sync.dma_start_transpose(wT[:, h * grp:(h + 1) * grp, :, :],
                                            wq_stage[:])
            wTs.append(wT)
        wgT, wuT, wdT = wTs
        # kb = swu * swd / 127, broadcast
        nc.vector.tensor_mul(svals[:, 6:7], svals[:, 2:3], svals[:, 4:5])
        nc.vector.tensor_scalar_mul(svals[:, 6:7], svals[:, 6:7], 1.0 / 127.0)
        nc.tensor.matmul(bcps[:, 6:7], ones_m[:], svals[:, 6:7],
                         start=True, stop=True)
        nc.scalar.copy(bcast[:, 6:7], bcps[:, 6:7])

        # ---------- main loop over token groups ----------
        # GEMM2 for chunk i is emitted after GEMM1 of chunk i+1 so the PE
        # stream never waits on chunk i's epilogue (sigmoid/quant/xbar) chain.
        def gemm2_flush(pend):
            hqT, inv_sx, m, ci = pend
            o1 = op_p.tile([128, 512], f32, tag="op", name="o1")
            o2 = op_p.tile([128, 256], f32, tag="op", name="o2")
            for mi in range(FT):
                nc.tensor.matmul(o1[:], hqT[:, mi, :], wdT[:, 0:4, mi, :],
                                 start=(mi == 0), stop=(mi == FT - 1))
                nc.tensor.matmul(o2[:], hqT[:, mi, :], wdT[:, 4:6, mi, :],
                                 start=(mi == 0), stop=(mi == FT - 1))
            s_out = small_p.tile([128, 1], f32, name="s_out")
            nc.vector.tensor_mul(s_out[:], inv_sx[:], m[:])
            nc.vector.tensor_mul(s_out[:], s_out[:], bcast[:, 6:7])
            ot = out_p.tile([128, D], f32, name="ot")
            nc.scalar.mul(ot[:, 0:512], o1[:], s_out[:])
            nc.scalar.mul(ot[:, 512:768], o2[:], s_out[:])
            nc.sync.dma_start(out_d[e, 128 * ci:128 * (ci + 1), :], ot[:])

        pend = None
        for g in range(CG):
            # x load + act quant + transpose, 4 c-chunks at a time
            xq_stage = xqs_p.tile([128, CPG, D], bf16, name="xq_stage")
            inv_sxs = []
            for ct in range(CPG):
                ci = g * CPG + ct
                xt = xld_p.tile([128, D], f32, name="xt")
                nc.sync.dma_start(xt[:], x_d[e, 128 * ci:128 * (ci + 1), :])
                amax = small_p.tile([128, 1], f32, name="amax")
                nc.vector.tensor_reduce(amax[:], xt[:], axis=AX, op=OP.max,
                                        apply_absolute_value=True)
                inv_sx = small_p.tile([128, 1], f32, name="inv_sx")
                nc.vector.tensor_scalar(inv_sx[:], amax[:], 1e-5, 1.0 / 127.0,
                                        OP.max, OP.mult)
                sx = small_p.tile([128, 1], f32, name="sx")
                nc.vector.reciprocal(sx[:], inv_sx[:])
                nc.vector.tensor_scalar(xt[:], xt[:], sx[:], MAGIC,
                                        OP.mult, OP.add)
                nc.gpsimd.tensor_scalar(xq_stage[:, ct, :], xt[:], MAGIC,
                                        None, OP.subtract)
                inv_sxs.append(inv_sx)
            xqT = xqT_p.tile([128, CPG, DT, 128], bf16, name="xqT")
            nc.sync.dma_start_transpose(xqT[:], xq_stage[:])

            for ct in range(CPG):
                ci = g * CPG + ct
                inv_sx = inv_sxs[ct]
                s_g = small_p.tile([128, 1], f32, name="s_g")
                nc.vector.tensor_mul(s_g[:], inv_sx[:], bcast[:, 0:1])
                t3 = t3_p.tile([128, F], f32, name="t3")
                for fg in range(FG):
                    gp = gp_p.tile([128, 512], f32, name="gp")
                    for k in range(DT):
                        nc.tensor.matmul(gp[:], xqT[:, ct, k, :],
                                         wgT[:, 4 * fg:4 * fg + 4, k, :],
                                         start=(k == 0), stop=(k == DT - 1))
                    sg = sg_p.tile([128, 512], f32, name="sg")
                    nc.scalar.activation(sg[:], gp[:], AF.Sigmoid,
                                         bias=0.0, scale=s_g[:])
                    up = up_p.tile([128, 512], f32, name="up")
                    for k in range(DT):
                        nc.tensor.matmul(up[:], xqT[:, ct, k, :],
                                         wuT[:, 4 * fg:4 * fg + 4, k, :],
                                         start=(k == 0), stop=(k == DT - 1))
                    nc.vector.tensor_mul(t3[:, 512 * fg:512 * (fg + 1)],
                                         sg[:], up[:])
                m = small_p.tile([128, 1], f32, name="m")
                nc.vector.tensor_reduce(m[:], t3[:], axis=AX, op=OP.max,
                                        apply_absolute_value=True)
                nc.vector.tensor_scalar_max(m[:], m[:], 1e-30)
                s2 = small_p.tile([128, 1], f32, name="s2")
                nc.vector.reciprocal(s2[:], m[:])
                nc.vector.tensor_scalar_mul(s2[:], s2[:], 127.0)
                hq = hq_p.tile([128, F], bf16, name="hq")
                for fg in range(FG):
                    t3s = t3[:, 512 * fg:512 * (fg + 1)]
                    nc.vector.tensor_scalar(t3s, t3s, s2[:], MAGIC,
                                            OP.mult, OP.add)
                    nc.gpsimd.tensor_scalar(hq[:, 512 * fg:512 * (fg + 1)],
                                            t3s, MAGIC, None, OP.subtract)
                hqT = hqT_p.tile([128, FT, 128], bf16, name="hqT")
                nc.sync.dma_start_transpose(hqT[:], hq[:])
                if pend is not None:
                    gemm2_flush(pend)
                pend = (hqT, inv_sx, m, ci)
        if pend is not None:
            gemm2_flush(pend)
            pend = None

    ctx.close()


def build(epc=EPC, C=C_FULL, num_devices=NCORES, loop_k=None):
    import concourse.mybir as mybir
    import concourse.tile as tile
    from concourse import bacc

    nc = bacc.Bacc("TRN2", target_bir_lowering=False, debug=False,
                   num_devices=num_devices)
    f32 = mybir.dt.float32
    x_d = nc.dram_tensor("x", [epc, C, D], f32, kind="ExternalInput").ap()
    wg_d = nc.dram_tensor("w_gate", [epc, F, D], f32, kind="ExternalInput").ap()
    wu_d = nc.dram_tensor("w_up", [epc, F, D], f32, kind="ExternalInput").ap()
    wd_d = nc.dram_tensor("w_down", [epc, D, F], f32, kind="ExternalInput").ap()
    out_d = nc.dram_tensor("out", [epc, C, D], f32, kind="ExternalOutput").ap()
    with tile.TileContext(nc) as tc:
        if loop_k is None:
            emit(tc, x_d, wg_d, wu_d, wd_d, out_d, epc, C)
        else:
            with tc.For_i(0, loop_k, 1):
                emit(tc, x_d, wg_d, wu_d, wd_d, out_d, epc, C)
    nc.compile()
    return nc


def kernel(x, w_gate, w_up, w_down, _trace=False):
    from concourse.bass_utils import run_bass_kernel_spmd

    key = "nc"
    if key not in _cache:
        _cache[key] = build()
    nc = _cache[key]

    in_maps = []
    for mcore in range(NCORES):
        sl = slice(mcore * EPC, (mcore + 1) * EPC)
        in_maps.append({
            "x": np.ascontiguousarray(x[sl], dtype=np.float32),
            "w_gate": np.ascontiguousarray(w_gate[sl], dtype=np.float32),
            "w_up": np.ascontiguousarray(w_up[sl], dtype=np.float32),
            "w_down": np.ascontiguousarray(w_down[sl], dtype=np.float32),
        })
    res = run_bass_kernel_spmd(nc, in_maps, core_ids=list(range(NCORES)),
                               trace=_trace)
    out = np.concatenate([res.results[m]["out"] for m in range(NCORES)], axis=0)
    if _trace:
        _cache["last_results"] = res
    return out.astype(np.float32, copy=False)


# revision 18
# speedup vs baseline: 3.2871x; 3.2871x over previous
"""Trainium2 Bass kernel for nn_BatchedBitNetFFN (BitNet b1.58 batched-expert FFN).

Math per expert (reference semantics reproduced exactly up to fp ulp noise):
  xq   = fake-quant(x): per-token int8 absmax  -> xq_int in [-127,127] (exact in bf16)
  wq   = ternary quant: per-expert scale mean|w| -> wq_int in {-1,0,1} (exact in bf16)
  gate = xq @ wq_g^T ; up = xq @ wq_u^T        (exact integer GEMMs, f32 PSUM accum)
  h    = sigmoid(gate*scale) * up
  hq   = fake-quant(h) per-token over F
  out  = (hq @ wq_d^T) * folded_scales

Sharding: expert-parallel, E=16 experts -> 2 experts on each of 8 NeuronCores,
no cross-core communication. Rounding uses the +/- 1.5*2^23 magic constant
(exact round-to-nearest-even, matching jnp.round). All transposed (K-on-
partition) operand layouts are produced by the HW xbar DMA-transpose on bf16.
"""

import numpy as np

E_FULL, C_FULL, D, F = 16, 4096, 768, 2048
NCORES = 8
EPC = E_FULL // NCORES  # experts per core
MAGIC = 12582912.0  # 1.5 * 2**23 -> exact RNE integer rounding via add/sub

_cache = {}


def emit(tc, x_d, wg_d, wu_d, wd_d, out_d, epc, C):
    import concourse.mybir as mybir

    nc = tc.nc
    f32 = mybir.dt.float32
    bf16 = mybir.dt.bfloat16
    AX = mybir.AxisListType.X
    OP = mybir.AluOpType
    AF = mybir.ActivationFunctionType

    DT = D // 128   # 6  d-chunks
    FT = F // 128   # 16 f-chunks
    FG = F // 512   # 4  f-groups (N=512)
    CG = C // 512   # c-groups per expert
    CPG = 4         # c-chunks (of 128 tokens) per c-group

    from contextlib import ExitStack
    ctx = ExitStack()
    tc._emit_ctx = ctx  # keep pools alive until TileContext exit

    const_p = ctx.enter_context(tc.tile_pool(name="const", bufs=1))
    wld_p = ctx.enter_context(tc.tile_pool(name="wld", bufs=3))
    stats_p = ctx.enter_context(tc.tile_pool(name="stats", bufs=3))
    small_p = ctx.enter_context(tc.tile_pool(name="small", bufs=8))
    wqs_p = ctx.enter_context(tc.tile_pool(name="wqs", bufs=1))
    wgT_p = ctx.enter_context(tc.tile_pool(name="wgTp", bufs=1))
    wuT_p = ctx.enter_context(tc.tile_pool(name="wuTp", bufs=1))
    wdT_p = ctx.enter_context(tc.tile_pool(name="wdTp", bufs=1))
    xld_p = ctx.enter_context(tc.tile_pool(name="xld", bufs=3))
    xqs_p = ctx.enter_context(tc.tile_pool(name="xqs", bufs=2))
    xqT_p = ctx.enter_context(tc.tile_pool(name="xqTp", bufs=2))
    sg_p = ctx.enter_context(tc.tile_pool(name="sgp", bufs=3))
    t3_p = ctx.enter_context(tc.tile_pool(name="t3p", bufs=3))
    hq_p = ctx.enter_context(tc.tile_pool(name="hqp", bufs=2))
    hqT_p = ctx.enter_context(tc.tile_pool(name="hqTp", bufs=3))
    out_p = ctx.enter_context(tc.tile_pool(name="outp", bufs=2))

    gp_p = ctx.enter_context(tc.tile_pool(name="gpp", bufs=2, space="PSUM"))
    up_p = ctx.enter_context(tc.tile_pool(name="upp", bufs=2, space="PSUM"))
    op_p = ctx.enter_context(tc.tile_pool(name="opp", bufs=3, space="PSUM"))
    bc_p = ctx.enter_context(tc.tile_pool(name="bcp", bufs=1, space="PSUM"))

    ones_k = const_p.tile([128, 1], f32, name="ones_k")
    nc.vector.memset(ones_k[:], 1.0)
    ones_m = const_p.tile([1, 128], f32, name="ones_m")
    nc.vector.memset(ones_m[:], 1.0)

    # weight descriptors: (dram, n 128-row tiles, row length, load tag)
    wspecs = [
        (wg_d, F // 128, D, "wta"),
        (wu_d, F // 128, D, "wta"),
        (wd_d, D // 128, F, "wtb"),
    ]

    for e in range(epc):
        # ---------- weight passes, per weight: scale then quantize ----------
        # bcast cols: 2j=sw_j  2j+1=rsw_j  6=kb(=swu*swd/127)
        bcps = bc_p.tile([128, 8], f32, tag="bcps", name=f"bcps{e}")
        svals = stats_p.tile([1, 8], f32, tag="svals", name=f"svals{e}")
        bcast = stats_p.tile([128, 8], f32, tag="bcast", name=f"bcast{e}")
        wTs = []
        for widx, (w_d, ntile, fd, wtag) in enumerate(wspecs):
            # pass 1: sw = clip(mean|w|, 1e-5)
            parts = stats_p.tile([128, ntile], f32, tag="parts", name=f"parts{e}_{widx}")
            for t in range(ntile):
                wt = wld_p.tile([128, fd], f32, tag=wtag, name=f"wt{wtag}",
                                bufs=(2 if widx == 2 else 3))
                nc.sync.dma_start(wt[:], w_d[e, 128 * t:128 * (t + 1), :])
                nc.scalar.activation(wt[:], wt[:], AF.Abs,
                                     accum_out=parts[:, t:t + 1])
            colsum = small_p.tile([128, 1], f32, name="colsum")
            nc.vector.tensor_reduce(colsum[:], parts[:], axis=AX, op=OP.add)
            # cross-partition sum -> [1,1]
            nc.tensor.matmul(bcps[0:1, 7:8], colsum[:], ones_k[:],
                             start=True, stop=True)
            sw = svals[:, 2 * widx:2 * widx + 1]
            nc.scalar.mul(sw, bcps[0:1, 7:8], 1.0 / (F * D))
            nc.vector.tensor_scalar_max(sw, sw, 1e-5)
            nc.vector.reciprocal(svals[:, 2 * widx + 1:2 * widx + 2], sw)
            # broadcast (sw, rsw) across partitions right away
            nc.tensor.matmul(bcps[:, 2 * widx:2 * widx + 2], ones_m[:],
                             svals[:, 2 * widx:2 * widx + 2],
                             start=True, stop=True)
            nc.scalar.copy(bcast[:, 2 * widx:2 * widx + 2],
                           bcps[:, 2 * widx:2 * widx + 2])

            # pass 2: ternary quantize -> bf16, xbar-transposed
            if widx < 2:
                wT = [wgT_p, wuT_p][widx].tile(
                    [128, FT, DT, 128], bf16, tag="wT",
                    name=f"w{'gu'[widx]}T{e}")
            else:
                wT = wdT_p.tile([128, DT, FT, 128], bf16, tag="wT",
                                name=f"wdT{e}")
            rsw = bcast[:, 2 * widx + 1:2 * widx + 2]
            grp = 2 if widx == 2 else 4
            for h in range(ntile // grp):
                wq_stage = wqs_p.tile([128, grp, fd], bf16, tag="wqs",
                                      name=f"wqs{e}_{widx}_{h}")
                for tt in range(grp):
                    t = h * grp + tt
                    wt = wld_p.tile([128, fd], f32, tag=wtag, name=f"wt{wtag}",
                                    bufs=(2 if widx == 2 else 3))
                    nc.sync.dma_start(wt[:], w_d[e, 128 * t:128 * (t + 1), :])
                    # round(clip(w/s,-1,1)) == clip(round(w/s),-1,1); RNE magic
                    nc.vector.tensor_scalar(wt[:], wt[:], rsw, MAGIC,
                                            OP.mult, OP.add)
                    nc.vector.tensor_scalar(wt[:], wt[:], MAGIC, 1.0,
                                            OP.subtract, OP.min)
                    nc.vector.tensor_scalar(wq_stage[:, tt, :], wt[:], -1.0,
                                            None, OP.max)
                nc.sync.dma_start_transpose(wT[:, h * grp:(h + 1) * grp, :, :],
                                            wq_stage[:])
            wTs.append(wT)
        wgT, wuT, wdT = wTs
        # kb = swu * swd / 127, broadcast
        nc.vector.tensor_mul(svals[:, 6:7], svals[:, 2:3], svals[:, 4:5])
        nc.vector.tensor_scalar_mul(svals[:, 6:7], svals[:, 6:7], 1.0 / 127.0)
        nc.tensor.matmul(bcps[:, 6:7], ones_m[:], svals[:, 6:7],
                         start=True, stop=True)
        nc.scalar.copy(bcast[:, 6:7], bcps[:, 6:7])

        # ---------- main loop over token groups ----------
        # GEMM2 for chunk i is emitted after GEMM1 of chunk i+1 so the PE
        # stream never waits on chunk i's epilogue (sigmoid/quant/xbar) chain.
        def gemm2_flush(pend):
            hqT, inv_sx, m, ci = pend
            o1 = op_p.tile([128, 512], f32, tag="op", name="o1")
            o2 = op_p.tile([128, 256], f32, tag="op", name="o2")
            for mi in range(FT):
                nc.tensor.matmul(o1[:], hqT[:, mi, :], wdT[:, 0:4, mi, :],
                                 start=(mi == 0), stop=(mi == FT - 1))
                nc.tensor.matmul(o2[:], hqT[:, mi, :], wdT[:, 4:6, mi, :],
                                 start=(mi == 0), stop=(mi == FT - 1))
            s_out = small_p.tile([128, 1], f32, name="s_out")
            nc.vector.tensor_mul(s_out[:], inv_sx[:], m[:])
            nc.vector.tensor_mul(s_out[:], s_out[:], bcast[:, 6:7])
            ot = out_p.tile([128, D], f32, name="ot")
            nc.scalar.mul(ot[:, 0:512], o1[:], s_out[:])
            nc.scalar.mul(ot[:, 512:768], o2[:], s_out[:])
            nc.sync.dma_start(out_d[e, 128 * ci:128 * (ci + 1), :], ot[:])

        pend = None
        for g in range(CG):
            # x load + act quant + transpose, 4 c-chunks at a time
            xq_stage = xqs_p.tile([128, CPG, D], bf16, name="xq_stage")
            inv_sxs = []
            for ct in range(CPG):
                ci = g * CPG + ct
                xt = xld_p.tile([128, D], f32, name="xt")
                nc.sync.dma_start(xt[:], x_d[e, 128 * ci:128 * (ci + 1), :])
                amax = small_p.tile([128, 1], f32, name="amax")
                nc.vector.tensor_reduce(amax[:], xt[:], axis=AX, op=OP.max,
                                        apply_absolute_value=True)
                inv_sx = small_p.tile([128, 1], f32, name="inv_sx")
                nc.vector.tensor_scalar(inv_sx[:], amax[:], 1e-5, 1.0 / 127.0,
                                        OP.max, OP.mult)
                sx = small_p.tile([128, 1], f32, name="sx")
                nc.vector.reciprocal(sx[:], inv_sx[:])
                nc.vector.tensor_scalar(xt[:], xt[:], sx[:], MAGIC,
                                        OP.mult, OP.add)
                nc.vector.tensor_scalar(xq_stage[:, ct, :], xt[:], MAGIC,
                                        None, OP.subtract)
                inv_sxs.append(inv_sx)
            xqT = xqT_p.tile([128, CPG, DT, 128], bf16, name="xqT")
            nc.sync.dma_start_transpose(xqT[:], xq_stage[:])

            for ct in range(CPG):
                ci = g * CPG + ct
                inv_sx = inv_sxs[ct]
                s_g = small_p.tile([128, 1], f32, name="s_g")
                nc.vector.tensor_mul(s_g[:], inv_sx[:], bcast[:, 0:1])
                t3 = t3_p.tile([128, F], f32, name="t3")
                for fg in range(FG):
                    gp = gp_p.tile([128, 512], f32, name="gp")
                    for k in range(DT):
                        nc.tensor.matmul(gp[:], xqT[:, ct, k, :],
                                         wgT[:, 4 * fg:4 * fg + 4, k, :],
                                         start=(k == 0), stop=(k == DT - 1))
                    sg = sg_p.tile([128, 512], f32, name="sg")
                    nc.scalar.activation(sg[:], gp[:], AF.Sigmoid,
                                         bias=0.0, scale=s_g[:])
                    up = up_p.tile([128, 512], f32, name="up")
                    for k in range(DT):
                        nc.tensor.matmul(up[:], xqT[:, ct, k, :],
                                         wuT[:, 4 * fg:4 * fg + 4, k, :],
                                         start=(k == 0), stop=(k == DT - 1))
                    nc.vector.tensor_mul(t3[:, 512 * fg:512 * (fg + 1)],
                                         sg[:], up[:])
                m = small_p.tile([128, 1], f32, name="m")
                nc.vector.tensor_reduce(m[:], t3[:], axis=AX, op=OP.max,
                                        apply_absolute_value=True)
                nc.vector.tensor_scalar_max(m[:], m[:], 1e-30)
                s2 = small_p.tile([128, 1], f32, name="s2")
                nc.vector.reciprocal(s2[:], m[:])
                nc.vector.tensor_scalar_mul(s2[:], s2[:], 127.0)
                hq = hq_p.tile([128, F], bf16, name="hq")
                for fg in range(FG):
                    t3s = t3[:, 512 * fg:512 * (fg + 1)]
                    nc.vector.tensor_scalar(t3s, t3s, s2[:], MAGIC,
                                            OP.mult, OP.add)
                    nc.vector.tensor_scalar(hq[:, 512 * fg:512 * (fg + 1)],
                                            t3s, MAGIC, None, OP.subtract)
                hqT = hqT_p.tile([128, FT, 128], bf16, name="hqT")
                nc.sync.dma_start_transpose(hqT[:], hq[:])
                if pend is not None:
                    gemm2_flush(pend)
                pend = (hqT, inv_sx, m, ci)
        if pend is not None:
            gemm2_flush(pend)
            pend = None

    ctx.close()


def build(epc=EPC, C=C_FULL, num_devices=NCORES, loop_k=None):
    import concourse.mybir as mybir
    import concourse.tile as tile
    from concourse import bacc

    nc = bacc.Bacc("TRN2", target_bir_lowering=False, debug=False,
                   num_devices=num_devices)
    f32 = mybir.dt.float32
    x_d = nc.dram_tensor("x", [epc, C, D], f32, kind="ExternalInput").ap()
    wg_d = nc.dram_tensor("w_gate", [epc, F, D], f32, kind="ExternalInput").ap()
    wu_d = nc.dram_tensor("w_up", [epc, F, D], f32, kind="ExternalInput").ap()
    wd_d = nc.dram_tensor("w_down", [epc, D, F], f32, kind="ExternalInput").ap()
    out_d = nc.dram_tensor("out", [epc, C, D], f32, kind="ExternalOutput").ap()
    with tile.TileContext(nc) as tc:
        if loop_k is None:
            emit(tc, x_d, wg_d, wu_d, wd_d, out_d, epc, C)
        else:
            with tc.For_i(0, loop_k, 1):
                emit(tc, x_d, wg_d, wu_d, wd_d, out_d, epc, C)
    nc.compile()
    return nc


def kernel(x, w_gate, w_up, w_down, _trace=False):
    from concourse.bass_utils import run_bass_kernel_spmd

    key = "nc"
    if key not in _cache:
        _cache[key] = build()
    nc = _cache[key]

    in_maps = []
    for mcore in range(NCORES):
        sl = slice(mcore * EPC, (mcore + 1) * EPC)
        in_maps.append({
            "x": np.ascontiguousarray(x[sl], dtype=np.float32),
            "w_gate": np.ascontiguousarray(w_gate[sl], dtype=np.float32),
            "w_up": np.ascontiguousarray(w_up[sl], dtype=np.float32),
            "w_down": np.ascontiguousarray(w_down[sl], dtype=np.float32),
        })
    res = run_bass_kernel_spmd(nc, in_maps, core_ids=list(range(NCORES)),
                               trace=_trace)
    out = np.concatenate([res.results[m]["out"] for m in range(NCORES)], axis=0)
    if _trace:
        _cache["last_results"] = res
    return out.astype(np.float32, copy=False)
